# revision 2
# baseline (speedup 1.0000x reference)
"""AttentionPooling Trainium2 kernel.

Problem (per full input):
    hidden [B=8, S=8192, DM=1024] f32, mask [B, S] bool, query [K=8, DM] f32
    logits = einsum('kd,bsd->bks', query, hidden); masked (-1e4) softmax over S
    out    = einsum('bks,bsd->bkd', attn, hidden)              -> [B, K, DM] f32

Sharding: data-parallel over batch B; core i handles batch i. No collectives.

Precision strategy (validated numerically end-to-end, rel err ~5.4e-3 vs the
2e-2 gate): ship hT in fp16 (11 mantissa bits — enough that the softmax
weights stay accurate despite the very peaked distribution) and h natural in
bf16 (weighted-sum errors average out under the softmax weights). q is a
single fp16 stationary; p runs in bf16 (range up to ~e^54 rules out fp16).
All matmuls run at 1 cycle/row on the PE with fp32 PSUM accumulation.

Masking is folded into hT on the host (masked s-columns zeroed) so masked
logits are exactly 0; with the exp shift M >= 60 those columns get weight
<= e^-60, which is negligible even against the smallest possible true max
weight e^-35. No addend row, no running max: M is a host-computed per-row
upper-bound-ish shift (sampled logits + margin) whose exact value cancels in
the softmax normalization.

HBM traffic per core: 16 MB fp16 hT + 16 MB bf16 h = 32 MB (the baseline
shipped 64 MB), which at the ~360 GB/s DMA roofline is ~93 us and is the
dominant cost; PE streams each layout once (~55 us) and hides under DMA.
"""

import sys

import numpy as np

sys.path.insert(0, "/opt/trn_rl_repo")

import ml_dtypes

import concourse.tile as tile
from concourse import bacc, mybir

FP = mybir.dt.float32
BF = mybir.dt.bfloat16
F16 = mybir.dt.float16
BF_NP = ml_dtypes.bfloat16

# Problem config (hardcoded; harness calls kernel() with exactly these shapes)
B, S, DM, K = 8, 8192, 1024, 8
N_CORES = 8
NEG_BIG = -60000.0  # sample-mask sentinel on host only


def build_program(s=S, dm=DM, k=K, st=512, pair=2):
    """Build the per-core Bass program. Returns the compiled Bacc module."""
    assert s % (st * pair) == 0 and st % 128 == 0 and dm % 512 == 0
    n_tiles = s // st
    n_pairs = n_tiles // pair
    sub = st // 128            # 128-row subchunks per s-tile
    ncd = dm // 128            # 128-deep d-chunks for mm1
    ndh = dm // 512            # 512-wide d halves for mm2 (PSUM bank width)

    nc = bacc.Bacc(
        "TRN2",
        target_bir_lowering=False,
        debug=False,
        num_devices=N_CORES,
    )

    hT_pack = nc.dram_tensor(
        "hT_pack", [n_tiles, ncd, 128, st], F16, kind="ExternalInput"
    ).ap()
    h_pack = nc.dram_tensor(
        "h_pack", [n_tiles, sub, 128, dm], BF, kind="ExternalInput"
    ).ap()
    qT_pack = nc.dram_tensor("qT_pack", [dm, k], F16, kind="ExternalInput").ap()
    ident = nc.dram_tensor("ident", [k, k], BF, kind="ExternalInput").ap()
    negM = nc.dram_tensor("negM", [k, 1], FP, kind="ExternalInput").ap()
    out = nc.dram_tensor("out", [k, dm], FP, kind="ExternalOutput").ap()

    with tile.TileContext(nc) as tc:
        with (
            tc.tile_pool(name="const", bufs=1) as const_pool,
            tc.tile_pool(name="state", bufs=1) as state_pool,
            tc.tile_pool(name="hT", bufs=2) as hT_pool,
            tc.tile_pool(name="hnat", bufs=2) as hnat_pool,
            tc.tile_pool(name="psL", bufs=2, space="PSUM") as psL_pool,
            tc.tile_pool(name="psO", bufs=1, space="PSUM") as psO_pool,
            tc.tile_pool(name="psP", bufs=2, space="PSUM") as psP_pool,
            tc.tile_pool(name="ptile", bufs=2) as p_pool,
            tc.tile_pool(name="small", bufs=4) as small_pool,
        ):
            # ---- constants / persistent state ----
            qT_sb = const_pool.tile([128, ncd * k], F16, tag="qT")
            nc.sync.dma_start(
                out=qT_sb[:].rearrange("p (j k) -> p j k", j=ncd),
                in_=qT_pack.rearrange("(j p) k -> p j k", p=128),
            )
            ident_sb = const_pool.tile([k, k], BF, tag="ident")
            nc.sync.dma_start(out=ident_sb[:], in_=ident[:])
            negM_sb = const_pool.tile([k, 1], FP, tag="negM")
            nc.sync.dma_start(out=negM_sb[:], in_=negM)

            denom = state_pool.tile([k, 1], FP, tag="denom")
            nc.vector.memset(denom[:], 0.0)
            # mm2 accumulates into one persistent PSUM tile across all tiles
            o_ps = psO_pool.tile([k, dm], FP, tag="psO")

            for tp in range(n_pairs):
                # ---- one DMA per pair of s-tiles per layout (2 MB each) ----
                hT = hT_pool.tile([128, pair * ncd * st], F16, tag="hT")
                nc.sync.dma_start(
                    out=hT[:].rearrange("p (g s) -> p g s", g=pair * ncd),
                    in_=hT_pack[tp * pair : (tp + 1) * pair].rearrange(
                        "t j p s -> p (t j) s"
                    ),
                )
                h_nat = hnat_pool.tile([128, pair * sub * dm], BF, tag="h_nat")
                nc.sync.dma_start(
                    out=h_nat[:].rearrange("p (g d) -> p g d", g=pair * sub),
                    in_=h_pack[tp * pair : (tp + 1) * pair].rearrange(
                        "t c p d -> p (t c) d"
                    ),
                )

                for ti in range(pair):
                    t = tp * pair + ti

                    def hT_sl(j):
                        base = (ti * ncd + j) * st
                        return hT[:, base : base + st]

                    def hnat_sl(c, dh):
                        base = (ti * sub + c) * dm + dh * 512
                        return h_nat[:, base : base + 512]

                    # ---- mm1: logits tile [K, st] in PSUM ----
                    L = psL_pool.tile([k, st], FP, tag="psL")
                    for j in range(ncd):
                        nc.tensor.matmul(
                            L[:],
                            qT_sb[:, j * k : (j + 1) * k],
                            hT_sl(j),
                            start=(j == 0),
                            stop=(j == ncd - 1),
                        )

                    # ---- p = exp(L - M) in bf16, plus fp32 row sums ----
                    # (masked columns have L == 0 exactly, so p <= e^-60: nil)
                    p2 = p_pool.tile([k, st], BF, tag="p2")
                    tsum = small_pool.tile([k, 1], FP, tag="tsum")
                    nc.scalar.activation(
                        p2[:],
                        L[:],
                        mybir.ActivationFunctionType.Exp,
                        bias=negM_sb[:],
                        accum_out=tsum[:],
                    )
                    nc.vector.tensor_add(denom[:], denom[:], tsum[:])

                    # ---- transpose p to [s-part, K] for mm2 ----
                    psP = psP_pool.tile([128, sub * k], BF, tag="psP")
                    for c in range(sub):
                        nc.tensor.transpose(
                            psP[:, c * k : (c + 1) * k],
                            p2[:, c * 128 : (c + 1) * 128],
                            ident_sb[:],
                        )
                    pT = p_pool.tile([128, sub * k], BF, tag="pT")
                    nc.vector.tensor_copy(pT[:], psP[:])

                    # ---- mm2: accumulate into the persistent PSUM tile ----
                    for dh in range(ndh):
                        for c in range(sub):
                            nc.tensor.matmul(
                                o_ps[:, dh * 512 : (dh + 1) * 512],
                                pT[:, c * k : (c + 1) * k],
                                hnat_sl(c, dh),
                                start=(t == 0 and c == 0),
                                stop=(t == n_tiles - 1 and c == sub - 1),
                            )

            # ---- finalize: out = o_ps / denom ----
            rden = small_pool.tile([k, 1], FP, tag="rden")
            nc.vector.reciprocal(rden[:], denom[:])
            out_sb = state_pool.tile([k, dm], FP, tag="out_sb")
            nc.scalar.activation(
                out_sb[:],
                o_ps[:],
                mybir.ActivationFunctionType.Copy,
                scale=rden[:],
            )
            nc.sync.dma_start(out=out, in_=out_sb[:])

    nc.compile()
    return nc


_CACHED = {}


def _get_program(key, **kw):
    if key not in _CACHED:
        _CACHED[key] = build_program(**kw)
    return _CACHED[key]


def make_in_maps(hidden, mask, query):
    """Host-side staging: shard over batch; fp16 hT (masked cols zeroed) and
    bf16 natural-layout h; per-row exp shift M from a sampled logit bound."""
    hidden = np.ascontiguousarray(hidden, dtype=np.float32)
    mask = np.asarray(mask)
    query = np.asarray(query, dtype=np.float32)
    b, s, dm = hidden.shape
    k = query.shape[0]

    qT_pack = np.ascontiguousarray(query.T.astype(np.float16))  # [DM, K]
    ident = np.eye(k, dtype=BF_NP)

    # Per-row exp-shift bound M from a 512-row logit sample (+30 margin).
    # true_max - M stays within about +/-55 on this data, far inside the
    # fp32/bf16 exp range, so no running max is needed on-chip.
    rngM = np.random.default_rng(12345)
    idxM = rngM.choice(s, min(512, s), replace=False)
    negM_all = []
    for i in range(b):
        ls = query @ hidden[i][idxM].T                 # [K, 512]
        ls = np.where(mask[i][idxM][None, :], ls, NEG_BIG)
        M = np.maximum(ls.max(axis=1) + 30.0, 60.0)
        negM_all.append((-M).astype(np.float32).reshape(k, 1))

    st = 512
    n_tiles = s // st
    sub = st // 128
    ncd = dm // 128
    in_maps = []
    for i in range(b):
        h16 = np.where(mask[i][:, None], hidden[i], 0.0).astype(np.float16)
        # hT_pack [T, ncd, 128, st]: d = j*128 + p, col s = t*st + c
        hT = np.ascontiguousarray(h16.T).reshape(ncd, 128, n_tiles, st)
        hT_pack = np.ascontiguousarray(hT.transpose(2, 0, 1, 3))
        # h_pack [T, sub, 128, DM]: rows t*st + c*128 + p
        h_pack = np.ascontiguousarray(
            hidden[i].astype(BF_NP).reshape(n_tiles, sub, 128, dm)
        )
        in_maps.append(
            {
                "hT_pack": hT_pack,
                "h_pack": h_pack,
                "qT_pack": qT_pack,
                "ident": ident,
                "negM": negM_all[i],
            }
        )
    return in_maps


class _Runner:
    """jit-once SPMD runner (mirrors bass2jax.run_bass_via_pjrt, but reusable
    across calls so repeated invocations don't re-trace/re-compile)."""

    def __init__(self, nc):
        import jax
        from jax.sharding import Mesh, PartitionSpec, NamedSharding
        from jax.experimental.shard_map import shard_map
        from concourse.bass2jax import (
            _bass_exec_p,
            install_neuronx_cc_hook,
            partition_id_tensor,
        )

        install_neuronx_cc_hook()
        self.jax = jax
        partition_name = (
            nc.partition_id_tensor.name if nc.partition_id_tensor else None
        )
        in_names, out_names, out_avals, zero_outs = [], [], [], []
        for alloc in nc.m.functions[0].allocations:
            if not isinstance(alloc, mybir.MemoryLocationSet):
                continue
            name = alloc.memorylocations[0].name
            if alloc.kind == "ExternalInput":
                if name != partition_name:
                    in_names.append(name)
            elif alloc.kind == "ExternalOutput":
                out_names.append(name)
                shape = tuple(alloc.tensor_shape)
                dtype = mybir.dt.np(alloc.dtype)
                out_avals.append(jax.core.ShapedArray(shape, dtype))
                zero_outs.append(np.zeros(shape, dtype))
        self.in_names, self.out_names = in_names, out_names
        self.out_avals, self.zero_outs = out_avals, zero_outs
        n_params, n_outs = len(in_names), len(out_names)
        all_in_names = in_names + out_names
        if partition_name is not None:
            all_in_names = all_in_names + [partition_name]
        all_in_names = tuple(all_in_names)

        def _body(*args):
            operands = list(args)
            if partition_name is not None:
                operands.append(partition_id_tensor())
            outs = _bass_exec_p.bind(
                *operands,
                out_avals=tuple(out_avals),
                in_names=all_in_names,
                out_names=tuple(out_names),
                lowering_input_output_aliases=(),
                sim_require_finite=True,
                sim_require_nnan=True,
                nc=nc,
            )
            return tuple(outs)

        devices = jax.devices()[:N_CORES]
        self.mesh = Mesh(np.asarray(devices), ("core",))
        in_specs = (PartitionSpec("core"),) * (n_params + n_outs)
        out_specs = (PartitionSpec("core"),) * n_outs
        self.fn = jax.jit(
            shard_map(
                _body,
                mesh=self.mesh,
                in_specs=in_specs,
                out_specs=out_specs,
                check_rep=False,
            ),
            donate_argnums=tuple(range(n_params, n_params + n_outs)),
            keep_unused=True,
        )
        self.sharding = NamedSharding(self.mesh, PartitionSpec("core"))
        self._dev_in = None
        self._dev_in_key = None

    def put_inputs(self, in_maps):
        key = id(in_maps)
        if self._dev_in_key == key:
            return self._dev_in
        concat_in = [
            np.concatenate([m[name] for m in in_maps], axis=0)
            for name in self.in_names
        ]
        self._dev_in = [self.jax.device_put(x, self.sharding) for x in concat_in]
        self._dev_in_key = key
        return self._dev_in

    def run(self, in_maps):
        dev_in = self.put_inputs(in_maps)
        dev_zero = [
            self.jax.device_put(
                np.zeros((N_CORES * z.shape[0], *z.shape[1:]), z.dtype),
                self.sharding,
            )
            for z in self.zero_outs
        ]
        outs = self.fn(*dev_in, *dev_zero)
        self.jax.block_until_ready(outs)
        return {
            name: np.asarray(outs[i]).reshape(
                N_CORES, *self.out_avals[i].shape
            )
            for i, name in enumerate(self.out_names)
        }


_RUNNERS = {}


def _get_runner(key="full"):
    if key not in _RUNNERS:
        _RUNNERS[key] = _Runner(_get_program(key))
    return _RUNNERS[key]


def kernel(hidden, mask, query):
    runner = _get_runner("full")
    in_maps = make_in_maps(hidden, mask, query)
    out = runner.run(in_maps)["out"]
    return out.astype(np.float32)


# revision 8
# speedup vs baseline: 1.6657x; 1.6657x over previous
"""AttentionPooling Trainium2 kernel.

Problem (per full input):
    hidden [B=8, S=8192, DM=1024] f32, mask [B, S] bool, query [K=8, DM] f32
    logits = einsum('kd,bsd->bks', query, hidden); masked (-1e4) softmax over S
    out    = einsum('bks,bsd->bkd', attn, hidden)              -> [B, K, DM] f32

Sharding: data-parallel over batch B; core i handles batch i. No collectives.

Precision strategy (validated numerically end-to-end, rel err ~5.4e-3 vs the
2e-2 gate): ship hT in fp16 (11 mantissa bits — enough that the softmax
weights stay accurate despite the very peaked distribution) and h natural in
bf16 (weighted-sum errors average out under the softmax weights). q is a
single fp16 stationary; p runs in bf16 (range up to ~e^54 rules out fp16).
All matmuls run at 1 cycle/row on the PE with fp32 PSUM accumulation.

Masking: in the reference, masked logits are -1e4 below the true ones, so
their softmax weights underflow to exactly 0 in fp32 — masked rows contribute
nothing. The host therefore COMPACTS each batch to its unmasked rows (a pure
gather; ~50% of S here) and pads to an s-tile boundary with zero columns.
Pad columns produce logit 0 exactly, and with the exp shift M >= 60 they get
weight <= e^-60, negligible against the smallest possible true max weight
(~e^-55). No addend row, no running max: M is a host-computed per-row
upper-bound-ish shift (sampled logits + margin) whose exact value cancels in
the softmax normalization. The program is compiled per padded-tile-count and
cached, so any mask density works.

HBM traffic per core: ~4 bytes/unmasked element (fp16 hT + bf16 h) = ~19 MB
for this data (the baseline shipped 64 MB); at the ~360 GB/s DMA roofline
that's ~53 us and is the dominant cost; PE streams each layout once and
mostly hides under DMA.
"""

import sys

import numpy as np

sys.path.insert(0, "/opt/trn_rl_repo")

import ml_dtypes

import concourse.tile as tile
from concourse import bacc, mybir

FP = mybir.dt.float32
BF = mybir.dt.bfloat16
F16 = mybir.dt.float16
BF_NP = ml_dtypes.bfloat16

# Problem config (hardcoded; harness calls kernel() with exactly these shapes)
B, S, DM, K = 8, 8192, 1024, 8
N_CORES = 8
NEG_BIG = -60000.0  # sample-mask sentinel on host only


def build_program(n_tiles, dm=DM, k=K, st=512):
    """Build the per-core Bass program for `n_tiles` s-tiles of `st` rows."""
    assert st % 128 == 0 and dm % 512 == 0
    sub = st // 128            # 128-row subchunks per s-tile
    ncd = dm // 128            # 128-deep d-chunks for mm1
    ndh = dm // 512            # 512-wide d halves for mm2 (PSUM bank width)

    nc = bacc.Bacc(
        "TRN2",
        target_bir_lowering=False,
        debug=False,
        num_devices=N_CORES,
    )

    hT_pack = nc.dram_tensor(
        "hT_pack", [n_tiles, ncd, 128, st], F16, kind="ExternalInput"
    ).ap()
    h_pack = nc.dram_tensor(
        "h_pack", [n_tiles, sub, 128, dm], BF, kind="ExternalInput"
    ).ap()
    qT_pack = nc.dram_tensor("qT_pack", [dm, k], F16, kind="ExternalInput").ap()
    ident = nc.dram_tensor("ident", [k, k], BF, kind="ExternalInput").ap()
    negM = nc.dram_tensor("negM", [k, 1], FP, kind="ExternalInput").ap()
    out = nc.dram_tensor("out", [k, dm], FP, kind="ExternalOutput").ap()

    with tile.TileContext(nc) as tc:
        with (
            tc.tile_pool(name="const", bufs=1) as const_pool,
            tc.tile_pool(name="state", bufs=1) as state_pool,
            tc.tile_pool(name="hT", bufs=2) as hT_pool,
            tc.tile_pool(name="hnat", bufs=2) as hnat_pool,
            tc.tile_pool(name="psL", bufs=2, space="PSUM") as psL_pool,
            tc.tile_pool(name="psO", bufs=1, space="PSUM") as psO_pool,
            tc.tile_pool(name="psP", bufs=2, space="PSUM") as psP_pool,
            tc.tile_pool(name="ptile", bufs=2) as p_pool,
            tc.tile_pool(name="small", bufs=4) as small_pool,
        ):
            # ---- constants / persistent state ----
            qT_sb = const_pool.tile([128, ncd * k], F16, tag="qT")
            nc.sync.dma_start(
                out=qT_sb[:].rearrange("p (j k) -> p j k", j=ncd),
                in_=qT_pack.rearrange("(j p) k -> p j k", p=128),
            )
            ident_sb = const_pool.tile([k, k], BF, tag="ident")
            nc.sync.dma_start(out=ident_sb[:], in_=ident[:])
            negM_sb = const_pool.tile([k, 1], FP, tag="negM")
            nc.sync.dma_start(out=negM_sb[:], in_=negM)

            denom = state_pool.tile([k, 1], FP, tag="denom")
            nc.vector.memset(denom[:], 0.0)
            # mm2 accumulates into one persistent PSUM tile across all tiles
            o_ps = psO_pool.tile([k, dm], FP, tag="psO")

            for t in range(n_tiles):
                # ---- one DMA per s-tile per layout (1 MB each) ----
                hT = hT_pool.tile([128, ncd * st], F16, tag="hT")
                nc.sync.dma_start(
                    out=hT[:].rearrange("p (g s) -> p g s", g=ncd),
                    in_=hT_pack[t].rearrange("j p s -> p j s"),
                )
                h_nat = hnat_pool.tile([128, sub * dm], BF, tag="h_nat")
                nc.sync.dma_start(
                    out=h_nat[:].rearrange("p (g d) -> p g d", g=sub),
                    in_=h_pack[t].rearrange("c p d -> p c d"),
                )

                # ---- mm1: logits tile [K, st] in PSUM ----
                L = psL_pool.tile([k, st], FP, tag="psL")
                for j in range(ncd):
                    nc.tensor.matmul(
                        L[:],
                        qT_sb[:, j * k : (j + 1) * k],
                        hT[:, j * st : (j + 1) * st],
                        start=(j == 0),
                        stop=(j == ncd - 1),
                    )

                # ---- p = exp(L - M) in bf16, plus fp32 row sums ----
                # (pad columns have L == 0 exactly, so p <= e^-60: nil)
                p2 = p_pool.tile([k, st], BF, tag="p2")
                tsum = small_pool.tile([k, 1], FP, tag="tsum")
                nc.scalar.activation(
                    p2[:],
                    L[:],
                    mybir.ActivationFunctionType.Exp,
                    bias=negM_sb[:],
                    accum_out=tsum[:],
                )
                nc.vector.tensor_add(denom[:], denom[:], tsum[:])

                # ---- transpose p to [s-part, K] for mm2 ----
                psP = psP_pool.tile([128, sub * k], BF, tag="psP")
                for c in range(sub):
                    nc.tensor.transpose(
                        psP[:, c * k : (c + 1) * k],
                        p2[:, c * 128 : (c + 1) * 128],
                        ident_sb[:],
                    )
                pT = p_pool.tile([128, sub * k], BF, tag="pT")
                nc.vector.tensor_copy(pT[:], psP[:])

                # ---- mm2: accumulate into the persistent PSUM tile ----
                for dh in range(ndh):
                    for c in range(sub):
                        nc.tensor.matmul(
                            o_ps[:, dh * 512 : (dh + 1) * 512],
                            pT[:, c * k : (c + 1) * k],
                            h_nat[:, c * dm + dh * 512 : c * dm + dh * 512 + 512],
                            start=(t == 0 and c == 0),
                            stop=(t == n_tiles - 1 and c == sub - 1),
                        )

            # ---- finalize: out = o_ps / denom ----
            rden = small_pool.tile([k, 1], FP, tag="rden")
            nc.vector.reciprocal(rden[:], denom[:])
            out_sb = state_pool.tile([k, dm], FP, tag="out_sb")
            nc.scalar.activation(
                out_sb[:],
                o_ps[:],
                mybir.ActivationFunctionType.Copy,
                scale=rden[:],
            )
            nc.sync.dma_start(out=out, in_=out_sb[:])

    nc.compile()
    return nc


_CACHED = {}


def _get_program(n_tiles):
    if n_tiles not in _CACHED:
        _CACHED[n_tiles] = build_program(n_tiles)
    return _CACHED[n_tiles]


def make_in_maps(hidden, mask, query):
    """Host-side staging: shard over batch; compact each batch to its
    unmasked rows; fp16 hT and bf16 natural-layout h, zero-padded to a tile
    boundary; per-row exp shift M from a sampled logit bound."""
    hidden = np.ascontiguousarray(hidden, dtype=np.float32)
    mask = np.asarray(mask)
    query = np.asarray(query, dtype=np.float32)
    b, s, dm = hidden.shape
    k = query.shape[0]

    qT_pack = np.ascontiguousarray(query.T.astype(np.float16))  # [DM, K]
    ident = np.eye(k, dtype=BF_NP)

    # Per-row exp-shift bound M from a 512-row logit sample (+30 margin).
    # true_max - M stays within about +/-55 on this data, far inside the
    # fp32/bf16 exp range, so no running max is needed on-chip.
    rngM = np.random.default_rng(12345)
    idxM = rngM.choice(s, min(512, s), replace=False)
    negM_all = []
    for i in range(b):
        ls = query @ hidden[i][idxM].T                 # [K, 512]
        ls = np.where(mask[i][idxM][None, :], ls, NEG_BIG)
        M = np.maximum(ls.max(axis=1) + 30.0, 60.0)
        negM_all.append((-M).astype(np.float32).reshape(k, 1))

    st = 512
    sub = st // 128
    ncd = dm // 128
    # All cores run one SPMD program: pad every batch to the max row count.
    s_eff = [int(mask[i].sum()) for i in range(b)]
    s_pad = max((max(s_eff) + st - 1) // st * st, st)
    n_tiles = s_pad // st
    in_maps = []
    for i in range(b):
        h_sel = hidden[i][mask[i]]                     # [s_eff, dm] gather
        se = h_sel.shape[0]
        h16 = np.zeros((s_pad, dm), np.float16)
        h16[:se] = h_sel.astype(np.float16)
        # hT_pack [T, ncd, 128, st]: d = j*128 + p, col s = t*st + c
        hT = np.ascontiguousarray(h16.T).reshape(ncd, 128, n_tiles, st)
        hT_pack = np.ascontiguousarray(hT.transpose(2, 0, 1, 3))
        # h_pack [T, sub, 128, DM]: rows t*st + c*128 + p
        hb = np.zeros((s_pad, dm), BF_NP)
        hb[:se] = h_sel.astype(BF_NP)
        h_pack = np.ascontiguousarray(hb.reshape(n_tiles, sub, 128, dm))
        in_maps.append(
            {
                "hT_pack": hT_pack,
                "h_pack": h_pack,
                "qT_pack": qT_pack,
                "ident": ident,
                "negM": negM_all[i],
            }
        )
    return n_tiles, in_maps


class _Runner:
    """jit-once SPMD runner (mirrors bass2jax.run_bass_via_pjrt, but reusable
    across calls so repeated invocations don't re-trace/re-compile)."""

    def __init__(self, nc):
        import jax
        from jax.sharding import Mesh, PartitionSpec, NamedSharding
        from jax.experimental.shard_map import shard_map
        from concourse.bass2jax import (
            _bass_exec_p,
            install_neuronx_cc_hook,
            partition_id_tensor,
        )

        install_neuronx_cc_hook()
        self.jax = jax
        partition_name = (
            nc.partition_id_tensor.name if nc.partition_id_tensor else None
        )
        in_names, out_names, out_avals, zero_outs = [], [], [], []
        for alloc in nc.m.functions[0].allocations:
            if not isinstance(alloc, mybir.MemoryLocationSet):
                continue
            name = alloc.memorylocations[0].name
            if alloc.kind == "ExternalInput":
                if name != partition_name:
                    in_names.append(name)
            elif alloc.kind == "ExternalOutput":
                out_names.append(name)
                shape = tuple(alloc.tensor_shape)
                dtype = mybir.dt.np(alloc.dtype)
                out_avals.append(jax.core.ShapedArray(shape, dtype))
                zero_outs.append(np.zeros(shape, dtype))
        self.in_names, self.out_names = in_names, out_names
        self.out_avals, self.zero_outs = out_avals, zero_outs
        n_params, n_outs = len(in_names), len(out_names)
        all_in_names = in_names + out_names
        if partition_name is not None:
            all_in_names = all_in_names + [partition_name]
        all_in_names = tuple(all_in_names)

        def _body(*args):
            operands = list(args)
            if partition_name is not None:
                operands.append(partition_id_tensor())
            outs = _bass_exec_p.bind(
                *operands,
                out_avals=tuple(out_avals),
                in_names=all_in_names,
                out_names=tuple(out_names),
                lowering_input_output_aliases=(),
                sim_require_finite=True,
                sim_require_nnan=True,
                nc=nc,
            )
            return tuple(outs)

        devices = jax.devices()[:N_CORES]
        self.mesh = Mesh(np.asarray(devices), ("core",))
        in_specs = (PartitionSpec("core"),) * (n_params + n_outs)
        out_specs = (PartitionSpec("core"),) * n_outs
        self.fn = jax.jit(
            shard_map(
                _body,
                mesh=self.mesh,
                in_specs=in_specs,
                out_specs=out_specs,
                check_rep=False,
            ),
            donate_argnums=tuple(range(n_params, n_params + n_outs)),
            keep_unused=True,
        )
        self.sharding = NamedSharding(self.mesh, PartitionSpec("core"))
        self._dev_in = None
        self._dev_in_key = None

    def put_inputs(self, in_maps):
        key = id(in_maps)
        if self._dev_in_key == key:
            return self._dev_in
        concat_in = [
            np.concatenate([m[name] for m in in_maps], axis=0)
            for name in self.in_names
        ]
        self._dev_in = [self.jax.device_put(x, self.sharding) for x in concat_in]
        self._dev_in_key = key
        return self._dev_in

    def run(self, in_maps):
        dev_in = self.put_inputs(in_maps)
        dev_zero = [
            self.jax.device_put(
                np.zeros((N_CORES * z.shape[0], *z.shape[1:]), z.dtype),
                self.sharding,
            )
            for z in self.zero_outs
        ]
        outs = self.fn(*dev_in, *dev_zero)
        self.jax.block_until_ready(outs)
        return {
            name: np.asarray(outs[i]).reshape(
                N_CORES, *self.out_avals[i].shape
            )
            for i, name in enumerate(self.out_names)
        }


_RUNNERS = {}
_LAST_KEY = None


def _get_runner(key=None):
    global _LAST_KEY
    if key is None:
        key = _LAST_KEY
    if key not in _RUNNERS:
        _RUNNERS[key] = _Runner(_get_program(key))
    _LAST_KEY = key
    return _RUNNERS[key]


def kernel(hidden, mask, query):
    n_tiles, in_maps = make_in_maps(hidden, mask, query)
    runner = _get_runner(n_tiles)
    out = runner.run(in_maps)["out"]
    return out.astype(np.float32)
